# revision 1
# baseline (speedup 1.0000x reference)
"""Trainium2 kernel for nn_LJCH1_34548716929306 (ragged_sequence).

Strategy (pure data-parallel over batch, per sharding hint):
  - The dominant cost is the fc0 projection: concat([featContent,
    featDistort, motionFeat]) [16,2048,4864] @ fc0_w.T [4864,128].
    That is ~637MB of activations -> memory-regime. It runs on the 8
    NeuronCores, 2 samples per core, as scores.T = wT.T @ xT with
    feature-major (K-major) layout prepared host-side so the device
    streams contiguous tiles with zero on-chip transposes. bf16
    operands, fp32 PSUM accumulation.
  - The BiGRU over T=2048 (H=32) and the masked multi-scale softmax
    head are tiny (~0.1% of FLOPs) and sequential; they run in fp32
    numpy on host.
"""

import numpy as np
import ml_dtypes
from concurrent.futures import ThreadPoolExecutor
from contextlib import ExitStack

import concourse.bass as bass
import concourse.bacc as bacc
import concourse.tile as tile
from concourse import mybir
from concourse.bass_utils import run_bass_kernel_spmd
from concourse.kernels.tile_matmul import matmul_tile_kernel

B, T = 16, 2048
D_CONTENT, D_DISTORT, D_MOTION = 4096, 512, 256
D = D_CONTENT + D_DISTORT + D_MOTION  # 4864
RED, H = 128, 32
N_CORES = 8
BL = B // N_CORES  # 2 samples per core
TIME_INTERVAL = 2
NEG = -1e9

_compiled = None


def _build_nc():
    nc = bacc.Bacc(
        "TRN2",
        target_bir_lowering=False,
        debug=False,
        enable_asserts=False,
        num_devices=N_CORES,
    )
    xT = nc.dram_tensor("xT", [D, BL * T], mybir.dt.bfloat16, kind="ExternalInput")
    wT = nc.dram_tensor("wT", [D, RED], mybir.dt.bfloat16, kind="ExternalInput")
    sT = nc.dram_tensor("sT", [RED, BL * T], mybir.dt.float32, kind="ExternalOutput")
    with tile.TileContext(nc) as tc:
        matmul_tile_kernel(tc, wT.ap(), xT.ap(), sT.ap())
    nc.compile()
    return nc


def _get_compiled():
    global _compiled
    if _compiled is None:
        _compiled = _build_nc()
    return _compiled


_runner = None


def _get_runner():
    """Build the sharded PJRT executable once and reuse it across calls.

    run_bass_kernel_spmd's axon path re-traces and re-jits the shard_map
    wrapper on every invocation (fresh closures defeat the jit cache);
    caching it here removes multi-second per-call overhead.
    """
    global _runner
    if _runner is not None:
        return _runner
    import jax
    from jax.sharding import Mesh, PartitionSpec
    from jax.experimental.shard_map import shard_map
    from concourse import bass2jax
    from concourse import mybir as _mybir

    nc = _get_compiled()
    bass2jax.install_neuronx_cc_hook()

    partition_name = nc.partition_id_tensor.name if nc.partition_id_tensor else None
    in_names, out_names, out_avals = [], [], []
    for alloc in nc.m.functions[0].allocations:
        if not isinstance(alloc, _mybir.MemoryLocationSet):
            continue
        name = alloc.memorylocations[0].name
        if alloc.kind == "ExternalInput":
            if name != partition_name:
                in_names.append(name)
        elif alloc.kind == "ExternalOutput":
            out_names.append(name)
            out_avals.append(
                jax.core.ShapedArray(tuple(alloc.tensor_shape), _mybir.dt.np(alloc.dtype))
            )
    n_params = len(in_names)
    n_outs = len(out_avals)
    all_in_names = list(in_names) + list(out_names)
    if partition_name is not None:
        all_in_names.append(partition_name)
    donate = tuple(range(n_params, n_params + n_outs))

    def _body(*args):
        operands = list(args)
        if partition_name is not None:
            operands.append(bass2jax.partition_id_tensor())
        outs = bass2jax._bass_exec_p.bind(
            *operands,
            out_avals=tuple(out_avals),
            in_names=tuple(all_in_names),
            out_names=tuple(out_names),
            lowering_input_output_aliases=(),
            sim_require_finite=True,
            sim_require_nnan=True,
            nc=nc,
        )
        return tuple(outs)

    devices = jax.devices()[:N_CORES]
    mesh = Mesh(np.asarray(devices), ("core",))
    in_specs = (PartitionSpec("core"),) * (n_params + n_outs)
    out_specs = (PartitionSpec("core"),) * n_outs
    sharded = jax.jit(
        shard_map(_body, mesh=mesh, in_specs=in_specs, out_specs=out_specs,
                  check_rep=False),
        donate_argnums=donate,
        keep_unused=True,
    )

    def run(in_maps):
        concat_in = [
            np.concatenate([np.asarray(m[name]) for m in in_maps], axis=0)
            for name in in_names
        ]
        concat_zeros = [
            np.zeros((N_CORES * a.shape[0], *a.shape[1:]), a.dtype) for a in out_avals
        ]
        out_arrs = sharded(*concat_in, *concat_zeros)
        return [
            {
                name: np.asarray(out_arrs[i]).reshape(N_CORES, *out_avals[i].shape)[c]
                for i, name in enumerate(out_names)
            }
            for c in range(N_CORES)
        ]

    _runner = run
    # expose pieces for external timing/inspection (test harness use)
    global _sharded, _mesh, _in_names_g, _out_names_g, _out_avals_g
    _sharded, _mesh = sharded, mesh
    _in_names_g, _out_names_g, _out_avals_g = in_names, out_names, out_avals
    return _runner


def _run_device(in_maps):
    try:
        return _get_runner()(in_maps)
    except Exception:
        return run_bass_kernel_spmd(_get_compiled(), in_maps, list(range(N_CORES))).results


def _sigmoid(x):
    return 1.0 / (1.0 + np.exp(-x))


def _gru_dir(gi, wh, bh, reverse):
    # gi: [T, B, 3H] precomputed input gates; returns ys [T, B, H]
    Tn, Bn, _ = gi.shape
    whT = wh.T.copy()  # [H, 3H]
    h = np.zeros((Bn, H), np.float32)
    ys = np.empty((Tn, Bn, H), np.float32)
    order = range(Tn - 1, -1, -1) if reverse else range(Tn)
    for t in order:
        g = gi[t]
        gh = h @ whT + bh
        i_r, i_z, i_n = g[:, :H], g[:, H : 2 * H], g[:, 2 * H :]
        h_r, h_z, h_n = gh[:, :H], gh[:, H : 2 * H], gh[:, 2 * H :]
        r = _sigmoid(i_r + h_r)
        z = _sigmoid(i_z + h_z)
        n = np.tanh(i_n + r * h_n)
        h = (1.0 - z) * n + z * h
        ys[t] = h
    return ys


def _conv1d_same(x, w):
    # cross-correlation with zero 'same' padding; x [B,T], w [k]
    k = w.shape[0]
    p = k // 2
    xp = np.pad(x, ((0, 0), (p, p)))
    out = np.zeros_like(x)
    for j in range(k):
        out += w[j] * xp[:, j : j + x.shape[1]]
    return out


def _make_in_maps(inputs):
    fC = np.asarray(inputs["featContent"], np.float32)
    fD = np.asarray(inputs["featDistort"], np.float32)
    mF = np.asarray(inputs["motionFeat"], np.float32)
    fc0_w = np.asarray(inputs["fc0_w"], np.float32)
    # Host-side layout prep: per-core feature-major bf16 [D, BL*T].
    wT_np = np.ascontiguousarray(fc0_w.T).astype(ml_dtypes.bfloat16)

    def build(c):
        sl = slice(c * BL, (c + 1) * BL)
        xT = np.empty((D, BL * T), ml_dtypes.bfloat16)
        xT[:D_CONTENT] = fC[sl].reshape(BL * T, D_CONTENT).T
        xT[D_CONTENT : D_CONTENT + D_DISTORT] = fD[sl].reshape(BL * T, D_DISTORT).T
        xT[D_CONTENT + D_DISTORT :] = mF[sl].reshape(BL * T, D_MOTION).T
        return {"xT": xT, "wT": wT_np}

    with ThreadPoolExecutor(N_CORES) as ex:
        return list(ex.map(build, range(N_CORES)))


def kernel(**inputs):
    inputLength = np.asarray(inputs["inputLength"])
    fc0_b = np.asarray(inputs["fc0_b"], np.float32)

    in_maps = _make_in_maps(inputs)
    results = _run_device(in_maps)

    scores = np.empty((B, T, RED), np.float32)
    for c in range(N_CORES):
        sT = results[c]["sT"]  # [RED, BL*T]
        scores[c * BL : (c + 1) * BL] = (
            sT.T.reshape(BL, T, RED).astype(np.float32)
        )
    scores += fc0_b

    # BiGRU (fp32 host)
    x_tbd = scores.transpose(1, 0, 2)  # [T,B,RED]
    gi_f = x_tbd @ np.asarray(inputs["gru_wi_f"], np.float32).T + np.asarray(
        inputs["gru_bi_f"], np.float32
    )
    gi_b = x_tbd @ np.asarray(inputs["gru_wi_b"], np.float32).T + np.asarray(
        inputs["gru_bi_b"], np.float32
    )
    yf = _gru_dir(gi_f, np.asarray(inputs["gru_wh_f"], np.float32),
                  np.asarray(inputs["gru_bh_f"], np.float32), reverse=False)
    yb = _gru_dir(gi_b, np.asarray(inputs["gru_wh_b"], np.float32),
                  np.asarray(inputs["gru_bh_b"], np.float32), reverse=True)
    outputs = np.concatenate([yf, yb], -1).transpose(1, 0, 2)  # [B,T,2H]

    q_w = np.asarray(inputs["q_w"], np.float32)
    q_b = np.asarray(inputs["q_b"], np.float32)
    q = (outputs @ q_w.T + q_b)[..., 0]  # [B,T]

    lengths = inputLength.astype(np.int64) - 2 * (TIME_INTERVAL // 2) - 1
    mask = np.arange(T)[None, :] < lengths[:, None]
    qm = np.where(mask, q, 0.0).astype(np.float32)

    total = np.zeros((B,), np.float32)
    for wk in ("w1", "w2", "w3"):
        w = np.asarray(inputs[wk], np.float32)
        logits = np.where(mask, _conv1d_same(qm, w), NEG).astype(np.float32)
        m = logits.max(-1, keepdims=True)
        e = np.exp(logits - m)
        sm = e / e.sum(-1, keepdims=True)
        total = total + (sm * qm).sum(-1)
    return (total / 3.0)[:, None].astype(np.float32)



# revision 6
# speedup vs baseline: 879.4193x; 879.4193x over previous
"""Trainium2 kernel for nn_LJCH1_34548716929306 (ragged_sequence).

Strategy (pure data-parallel over batch, per sharding hint):
  - The dominant cost is the fc0 projection: concat([featContent,
    featDistort, motionFeat]) [16,2048,4864] @ fc0_w.T [4864,128].
    That is ~637MB of activations -> memory-regime. It runs on the 8
    NeuronCores, 2 samples per core, as a custom tile kernel computing
    sT = wH.T @ xH with K=4864 on partitions.  Host-side the activations
    are pre-tiled to [128, 8, 38, 512] fp8e4m3 (partition-major blocks)
    so every DMA is a fully contiguous 19KB-per-partition stream, and
    the weight to [128, 38, 128] bf16.  The PE runs mixed-precision
    matmuls (bf16 stationary weights x fp8 moving activations --
    verified bit-exact on HW), accumulating 38 K-subtiles into one PSUM
    bank per 512-wide output tile; fp32 PSUM, bf16 scores out.
    fp8 activations halve the HBM traffic (the kernel is DMA-bound);
    keeping the weights bf16 keeps the end-to-end error at 1.6e-2
    (vs 2.3e-2 with fp8 weights, over the 2e-2 gate).
  - The BiGRU over T=2048 (H=32) and the masked multi-scale softmax
    head are tiny (~0.1% of FLOPs) and sequential; they run in fp32
    numpy on host.

Timing note: the axon tunnel has a ~70ms fixed round-trip latency that
dwarfs the device time of a single exec.  _build_nc(unroll, loop_reps)
therefore can also build a variant whose body repeats unroll*loop_reps
times inside one NEFF (hardware For_i loop over an unrolled body);
test.py differences two trip counts to isolate the true steady-state
per-iteration device time (the RTT and launch overhead cancel).
"""

import numpy as np
import ml_dtypes
from concurrent.futures import ThreadPoolExecutor

import concourse.bass as bass
import concourse.bacc as bacc
import concourse.tile as tile
from concourse import mybir

B, T = 16, 2048
D_CONTENT, D_DISTORT, D_MOTION = 4096, 512, 256
D = D_CONTENT + D_DISTORT + D_MOTION  # 4864
RED, H = 128, 32
N_CORES = 8
BL = B // N_CORES  # 2 samples per core
BLT = BL * T  # 4096
KO = D // 128  # 38 K-subtiles
NT = BLT // 512  # 8 output column tiles
TIME_INTERVAL = 2
NEG = -1e9


def _build_nc(unroll=1, loop_reps=0):
    """Build the fc0-projection bass kernel.

    unroll: how many copies of the kernel body are emitted.
    loop_reps: if >0, wrap the unrolled body in a hardware For_i loop
      with this trip count (used by test.py for RTT-cancelling timing).
    The computed output is identical regardless (same inputs -> same
    sT each repetition).
    """
    nc = bacc.Bacc(
        "TRN2",
        target_bir_lowering=False,
        debug=False,
        enable_asserts=False,
        num_devices=N_CORES,
    )
    xH = nc.dram_tensor("xH", [128, NT, KO, 512], mybir.dt.float8e4, kind="ExternalInput")
    wH = nc.dram_tensor("wH", [128, KO, RED], mybir.dt.bfloat16, kind="ExternalInput")
    sT = nc.dram_tensor("sT", [RED, BLT], mybir.dt.bfloat16, kind="ExternalOutput")

    with tile.TileContext(nc) as tc:
        with tc.tile_pool(name="w", bufs=2) as wpool, \
             tc.tile_pool(name="x", bufs=3) as xpool, \
             tc.tile_pool(name="o", bufs=3) as opool, \
             tc.tile_pool(name="ps", bufs=2, space="PSUM") as pspool:

            def body():
                w_sb = wpool.tile([128, KO, RED], mybir.dt.bfloat16, tag="w")
                nc.sync.dma_start(w_sb[:], wH.ap())
                for n in range(NT):
                    x_sb = xpool.tile([128, KO, 512], mybir.dt.float8e4, tag="x")
                    nc.sync.dma_start(x_sb[:], xH.ap()[:, n])
                    ps = pspool.tile([RED, 512], mybir.dt.float32, tag="ps")
                    for k in range(KO):
                        nc.tensor.matmul(
                            ps[:], w_sb[:, k], x_sb[:, k],
                            start=(k == 0), stop=(k == KO - 1),
                        )
                    o_sb = opool.tile([RED, 512], mybir.dt.bfloat16, tag="o")
                    nc.vector.tensor_copy(o_sb[:], ps[:])
                    nc.sync.dma_start(sT.ap()[:, bass.ts(n, 512)], o_sb[:])

            if loop_reps:
                with tc.For_i(0, loop_reps, 1,
                              hint_engines=(mybir.EngineType.PE,)):
                    for _ in range(unroll):
                        body()
            else:
                for _ in range(unroll):
                    body()
    nc.compile()
    return nc


_runners = {}


def _get_runner(key, build_fn):
    """Build + cache the sharded PJRT executable for a bass program.

    run_bass_kernel_spmd's axon path re-traces and re-jits the shard_map
    wrapper on every invocation (fresh closures defeat the jit cache);
    caching it here removes multi-second per-call overhead.
    """
    if key in _runners:
        return _runners[key]
    import jax
    from jax.sharding import Mesh, PartitionSpec
    from jax.experimental.shard_map import shard_map
    from concourse import bass2jax
    from concourse import mybir as _mybir

    nc = build_fn()
    bass2jax.install_neuronx_cc_hook()

    partition_name = nc.partition_id_tensor.name if nc.partition_id_tensor else None
    in_names, out_names, out_avals = [], [], []
    for alloc in nc.m.functions[0].allocations:
        if not isinstance(alloc, _mybir.MemoryLocationSet):
            continue
        name = alloc.memorylocations[0].name
        if alloc.kind == "ExternalInput":
            if name != partition_name:
                in_names.append(name)
        elif alloc.kind == "ExternalOutput":
            out_names.append(name)
            out_avals.append(
                jax.core.ShapedArray(tuple(alloc.tensor_shape), _mybir.dt.np(alloc.dtype))
            )
    n_params = len(in_names)
    n_outs = len(out_avals)
    all_in_names = list(in_names) + list(out_names)
    if partition_name is not None:
        all_in_names.append(partition_name)
    donate = tuple(range(n_params, n_params + n_outs))

    def _body(*args):
        operands = list(args)
        if partition_name is not None:
            operands.append(bass2jax.partition_id_tensor())
        outs = bass2jax._bass_exec_p.bind(
            *operands,
            out_avals=tuple(out_avals),
            in_names=tuple(all_in_names),
            out_names=tuple(out_names),
            lowering_input_output_aliases=(),
            sim_require_finite=True,
            sim_require_nnan=True,
            nc=nc,
        )
        return tuple(outs)

    devices = jax.devices()[:N_CORES]
    mesh = Mesh(np.asarray(devices), ("core",))
    in_specs = (PartitionSpec("core"),) * (n_params + n_outs)
    out_specs = (PartitionSpec("core"),) * n_outs
    sharded = jax.jit(
        shard_map(_body, mesh=mesh, in_specs=in_specs, out_specs=out_specs,
                  check_rep=False),
        donate_argnums=donate,
        keep_unused=True,
    )

    handle = {
        "nc": nc, "sharded": sharded, "mesh": mesh,
        "in_names": in_names, "out_names": out_names, "out_avals": out_avals,
    }
    _runners[key] = handle
    return handle


def _run_handle(handle, in_maps):
    in_names = handle["in_names"]
    out_names = handle["out_names"]
    out_avals = handle["out_avals"]
    concat_in = [
        np.concatenate([np.asarray(m[name]) for m in in_maps], axis=0)
        for name in in_names
    ]
    concat_zeros = [
        np.zeros((N_CORES * a.shape[0], *a.shape[1:]), a.dtype) for a in out_avals
    ]
    out_arrs = handle["sharded"](*concat_in, *concat_zeros)
    return [
        {
            name: np.asarray(out_arrs[i]).reshape(N_CORES, *out_avals[i].shape)[c]
            for i, name in enumerate(out_names)
        }
        for c in range(N_CORES)
    ]


def _run_device(in_maps):
    handle = _get_runner("main", _build_nc)
    try:
        return _run_handle(handle, in_maps)
    except Exception:
        from concourse.bass_utils import run_bass_kernel_spmd
        return run_bass_kernel_spmd(handle["nc"], in_maps, list(range(N_CORES))).results


def _sigmoid(x):
    return 1.0 / (1.0 + np.exp(-x))


def _gru_dir(gi, wh, bh, reverse):
    # gi: [T, B, 3H] precomputed input gates; returns ys [T, B, H]
    Tn, Bn, _ = gi.shape
    whT = wh.T.copy()  # [H, 3H]
    h = np.zeros((Bn, H), np.float32)
    ys = np.empty((Tn, Bn, H), np.float32)
    order = range(Tn - 1, -1, -1) if reverse else range(Tn)
    for t in order:
        g = gi[t]
        gh = h @ whT + bh
        i_r, i_z, i_n = g[:, :H], g[:, H : 2 * H], g[:, 2 * H :]
        h_r, h_z, h_n = gh[:, :H], gh[:, H : 2 * H], gh[:, 2 * H :]
        r = _sigmoid(i_r + h_r)
        z = _sigmoid(i_z + h_z)
        n = np.tanh(i_n + r * h_n)
        h = (1.0 - z) * n + z * h
        ys[t] = h
    return ys


def _conv1d_same(x, w):
    # cross-correlation with zero 'same' padding; x [B,T], w [k]
    k = w.shape[0]
    p = k // 2
    xp = np.pad(x, ((0, 0), (p, p)))
    out = np.zeros_like(x)
    for j in range(k):
        out += w[j] * xp[:, j : j + x.shape[1]]
    return out


_F8 = getattr(ml_dtypes, "float8_e4m3fn", None) or ml_dtypes.float8_e4m3


def _make_in_maps(inputs):
    fC = np.asarray(inputs["featContent"], np.float32)
    fD = np.asarray(inputs["featDistort"], np.float32)
    mF = np.asarray(inputs["motionFeat"], np.float32)
    fc0_w = np.asarray(inputs["fc0_w"], np.float32)
    # wH[p, ko, m] = fc0_w[m, ko*128+p]
    wH_np = np.ascontiguousarray(
        fc0_w.reshape(RED, KO, 128).transpose(2, 1, 0)
    ).astype(ml_dtypes.bfloat16)

    def build(c):
        sl = slice(c * BL, (c + 1) * BL)
        # xH[p, n, ko, c] = concat_feat[n*512+c (flat b*T+t), ko*128+p]
        # Values are quantized with e4m3fn (OCP) semantics -- probed to be
        # the HW's fp8e4 interpretation -- then bit-reinterpreted to the
        # ml_dtypes.float8_e4m3 container mybir.dt.np(float8e4) expects.
        xH = np.empty((128, NT, KO, 512), _F8)
        fc = fC[sl].reshape(BLT, D_CONTENT).astype(_F8)
        xH[:, :, 0:32, :] = fc.reshape(NT, 512, 32, 128).transpose(3, 0, 2, 1)
        fd = fD[sl].reshape(BLT, D_DISTORT).astype(_F8)
        xH[:, :, 32:36, :] = fd.reshape(NT, 512, 4, 128).transpose(3, 0, 2, 1)
        mf = mF[sl].reshape(BLT, D_MOTION).astype(_F8)
        xH[:, :, 36:38, :] = mf.reshape(NT, 512, 2, 128).transpose(3, 0, 2, 1)
        return {"xH": xH.view(ml_dtypes.float8_e4m3), "wH": wH_np}

    with ThreadPoolExecutor(N_CORES) as ex:
        return list(ex.map(build, range(N_CORES)))


def kernel(**inputs):
    inputLength = np.asarray(inputs["inputLength"])
    fc0_b = np.asarray(inputs["fc0_b"], np.float32)

    in_maps = _make_in_maps(inputs)
    results = _run_device(in_maps)

    scores = np.empty((B, T, RED), np.float32)
    for c in range(N_CORES):
        sT = results[c]["sT"]  # [RED, BLT]
        scores[c * BL : (c + 1) * BL] = (
            sT.T.reshape(BL, T, RED).astype(np.float32)
        )
    scores += fc0_b

    # BiGRU (fp32 host)
    x_tbd = scores.transpose(1, 0, 2)  # [T,B,RED]
    gi_f = x_tbd @ np.asarray(inputs["gru_wi_f"], np.float32).T + np.asarray(
        inputs["gru_bi_f"], np.float32
    )
    gi_b = x_tbd @ np.asarray(inputs["gru_wi_b"], np.float32).T + np.asarray(
        inputs["gru_bi_b"], np.float32
    )
    yf = _gru_dir(gi_f, np.asarray(inputs["gru_wh_f"], np.float32),
                  np.asarray(inputs["gru_bh_f"], np.float32), reverse=False)
    yb = _gru_dir(gi_b, np.asarray(inputs["gru_wh_b"], np.float32),
                  np.asarray(inputs["gru_bh_b"], np.float32), reverse=True)
    outputs = np.concatenate([yf, yb], -1).transpose(1, 0, 2)  # [B,T,2H]

    q_w = np.asarray(inputs["q_w"], np.float32)
    q_b = np.asarray(inputs["q_b"], np.float32)
    q = (outputs @ q_w.T + q_b)[..., 0]  # [B,T]

    lengths = inputLength.astype(np.int64) - 2 * (TIME_INTERVAL // 2) - 1
    mask = np.arange(T)[None, :] < lengths[:, None]
    qm = np.where(mask, q, 0.0).astype(np.float32)

    total = np.zeros((B,), np.float32)
    for wk in ("w1", "w2", "w3"):
        w = np.asarray(inputs[wk], np.float32)
        logits = np.where(mask, _conv1d_same(qm, w), NEG).astype(np.float32)
        m = logits.max(-1, keepdims=True)
        e = np.exp(logits - m)
        sm = e / e.sum(-1, keepdims=True)
        total = total + (sm * qm).sum(-1)
    return (total / 3.0)[:, None].astype(np.float32)


# revision 7
# speedup vs baseline: 932.0477x; 1.0598x over previous
"""Trainium2 kernel for nn_LJCH1_34548716929306 (ragged_sequence).

Strategy (pure data-parallel over batch, per sharding hint):
  - The dominant cost is the fc0 projection: concat([featContent,
    featDistort, motionFeat]) [16,2048,4864] @ fc0_w.T [4864,128].
    That is ~637MB of activations -> memory-regime. It runs on the 8
    NeuronCores, 2 samples per core, as a custom tile kernel computing
    sT = wH.T @ xH with K=4864 on partitions.  Host-side the activations
    are pre-tiled to [128, 8, 38, 512] fp8e4m3 (partition-major blocks)
    so every DMA is a fully contiguous 19KB-per-partition stream, and
    the weight to [128, 38, 128] bf16.  The PE runs mixed-precision
    matmuls (bf16 stationary weights x fp8 moving activations --
    verified bit-exact on HW), accumulating 38 K-subtiles into one PSUM
    bank per 512-wide output tile; fp32 PSUM, bf16 scores out.
    fp8 activations halve the HBM traffic (the kernel is DMA-bound);
    keeping the weights bf16 keeps the end-to-end error at 1.6e-2
    (vs 2.3e-2 with fp8 weights, over the 2e-2 gate).
  - The BiGRU over T=2048 (H=32) and the masked multi-scale softmax
    head are tiny (~0.1% of FLOPs) and sequential; they run in fp32
    numpy on host.

Timing note: the axon tunnel has a ~70ms fixed round-trip latency that
dwarfs the device time of a single exec.  _build_nc(unroll, loop_reps)
therefore can also build a variant whose body repeats unroll*loop_reps
times inside one NEFF (hardware For_i loop over an unrolled body);
test.py differences two trip counts to isolate the true steady-state
per-iteration device time (the RTT and launch overhead cancel).
"""

import numpy as np
import ml_dtypes
from concurrent.futures import ThreadPoolExecutor

import concourse.bass as bass
import concourse.bacc as bacc
import concourse.tile as tile
from concourse import mybir

B, T = 16, 2048
D_CONTENT, D_DISTORT, D_MOTION = 4096, 512, 256
D = D_CONTENT + D_DISTORT + D_MOTION  # 4864
RED, H = 128, 32
N_CORES = 8
BL = B // N_CORES  # 2 samples per core
BLT = BL * T  # 4096
KO = D // 128  # 38 K-subtiles
NT = BLT // 512  # 8 output column tiles
TIME_INTERVAL = 2
NEG = -1e9


def _build_nc(unroll=1, loop_reps=0):
    """Build the fc0-projection bass kernel.

    unroll: how many copies of the kernel body are emitted.
    loop_reps: if >0, wrap the unrolled body in a hardware For_i loop
      with this trip count (used by test.py for RTT-cancelling timing).
    The computed output is identical regardless (same inputs -> same
    sT each repetition).
    """
    nc = bacc.Bacc(
        "TRN2",
        target_bir_lowering=False,
        debug=False,
        enable_asserts=False,
        num_devices=N_CORES,
    )
    xH = nc.dram_tensor("xH", [128, NT, KO, 512], mybir.dt.float8e4, kind="ExternalInput")
    wH = nc.dram_tensor("wH", [128, KO, RED], mybir.dt.bfloat16, kind="ExternalInput")
    sT = nc.dram_tensor("sT", [RED, BLT], mybir.dt.bfloat16, kind="ExternalOutput")

    with tile.TileContext(nc) as tc:
        with tc.tile_pool(name="w", bufs=2) as wpool, \
             tc.tile_pool(name="x", bufs=3) as xpool, \
             tc.tile_pool(name="o", bufs=3) as opool, \
             tc.tile_pool(name="ps", bufs=2, space="PSUM") as pspool:

            def body():
                w_sb = wpool.tile([128, KO, RED], mybir.dt.bfloat16, tag="w")
                nc.sync.dma_start(w_sb[:], wH.ap())
                for n in range(NT):
                    x_sb = xpool.tile([128, KO, 512], mybir.dt.float8e4, tag="x")
                    nc.sync.dma_start(x_sb[:], xH.ap()[:, n])
                    ps = pspool.tile([RED, 512], mybir.dt.float32, tag="ps")
                    for k in range(KO):
                        nc.tensor.matmul(
                            ps[:], w_sb[:, k], x_sb[:, k],
                            start=(k == 0), stop=(k == KO - 1),
                        )
                    o_sb = opool.tile([RED, 512], mybir.dt.bfloat16, tag="o")
                    nc.vector.tensor_copy(o_sb[:], ps[:])
                    nc.sync.dma_start(sT.ap()[:, bass.ts(n, 512)], o_sb[:])

            if loop_reps:
                with tc.For_i(0, loop_reps, 1,
                              hint_engines=(mybir.EngineType.PE,)):
                    for _ in range(unroll):
                        body()
            else:
                for _ in range(unroll):
                    body()
    nc.compile()
    return nc


_runners = {}


def _get_runner(key, build_fn):
    """Build + cache the sharded PJRT executable for a bass program.

    run_bass_kernel_spmd's axon path re-traces and re-jits the shard_map
    wrapper on every invocation (fresh closures defeat the jit cache);
    caching it here removes multi-second per-call overhead.
    """
    if key in _runners:
        return _runners[key]
    import jax
    from jax.sharding import Mesh, PartitionSpec
    from jax.experimental.shard_map import shard_map
    from concourse import bass2jax
    from concourse import mybir as _mybir

    nc = build_fn()
    bass2jax.install_neuronx_cc_hook()

    partition_name = nc.partition_id_tensor.name if nc.partition_id_tensor else None
    in_names, out_names, out_avals = [], [], []
    for alloc in nc.m.functions[0].allocations:
        if not isinstance(alloc, _mybir.MemoryLocationSet):
            continue
        name = alloc.memorylocations[0].name
        if alloc.kind == "ExternalInput":
            if name != partition_name:
                in_names.append(name)
        elif alloc.kind == "ExternalOutput":
            out_names.append(name)
            out_avals.append(
                jax.core.ShapedArray(tuple(alloc.tensor_shape), _mybir.dt.np(alloc.dtype))
            )
    n_params = len(in_names)
    n_outs = len(out_avals)
    all_in_names = list(in_names) + list(out_names)
    if partition_name is not None:
        all_in_names.append(partition_name)
    donate = tuple(range(n_params, n_params + n_outs))

    def _body(*args):
        operands = list(args)
        if partition_name is not None:
            operands.append(bass2jax.partition_id_tensor())
        outs = bass2jax._bass_exec_p.bind(
            *operands,
            out_avals=tuple(out_avals),
            in_names=tuple(all_in_names),
            out_names=tuple(out_names),
            lowering_input_output_aliases=(),
            sim_require_finite=True,
            sim_require_nnan=True,
            nc=nc,
        )
        return tuple(outs)

    devices = jax.devices()[:N_CORES]
    mesh = Mesh(np.asarray(devices), ("core",))
    in_specs = (PartitionSpec("core"),) * (n_params + n_outs)
    out_specs = (PartitionSpec("core"),) * n_outs
    sharded = jax.jit(
        shard_map(_body, mesh=mesh, in_specs=in_specs, out_specs=out_specs,
                  check_rep=False),
        donate_argnums=donate,
        keep_unused=True,
    )

    handle = {
        "nc": nc, "sharded": sharded, "mesh": mesh,
        "in_names": in_names, "out_names": out_names, "out_avals": out_avals,
    }
    _runners[key] = handle
    return handle


def _run_handle(handle, in_maps):
    in_names = handle["in_names"]
    out_names = handle["out_names"]
    out_avals = handle["out_avals"]
    concat_in = [
        np.concatenate([np.asarray(m[name]) for m in in_maps], axis=0)
        for name in in_names
    ]
    concat_zeros = [
        np.zeros((N_CORES * a.shape[0], *a.shape[1:]), a.dtype) for a in out_avals
    ]
    out_arrs = handle["sharded"](*concat_in, *concat_zeros)
    return [
        {
            name: np.asarray(out_arrs[i]).reshape(N_CORES, *out_avals[i].shape)[c]
            for i, name in enumerate(out_names)
        }
        for c in range(N_CORES)
    ]


def _run_device(in_maps):
    import time as _time
    handle = _get_runner("main", _build_nc)
    # Retry transient runtime flakes (e.g. NRT_EXEC_UNIT_UNRECOVERABLE --
    # observed once; the terminal recovers the core within seconds).
    last = None
    for attempt in range(3):
        try:
            return _run_handle(handle, in_maps)
        except Exception as e:
            last = e
            _time.sleep(3.0 * (attempt + 1))
    try:
        from concourse.bass_utils import run_bass_kernel_spmd
        return run_bass_kernel_spmd(handle["nc"], in_maps, list(range(N_CORES))).results
    except Exception:
        raise last


def _sigmoid(x):
    return 1.0 / (1.0 + np.exp(-x))


def _gru_dir(gi, wh, bh, reverse):
    # gi: [T, B, 3H] precomputed input gates; returns ys [T, B, H]
    Tn, Bn, _ = gi.shape
    whT = wh.T.copy()  # [H, 3H]
    h = np.zeros((Bn, H), np.float32)
    ys = np.empty((Tn, Bn, H), np.float32)
    order = range(Tn - 1, -1, -1) if reverse else range(Tn)
    for t in order:
        g = gi[t]
        gh = h @ whT + bh
        i_r, i_z, i_n = g[:, :H], g[:, H : 2 * H], g[:, 2 * H :]
        h_r, h_z, h_n = gh[:, :H], gh[:, H : 2 * H], gh[:, 2 * H :]
        r = _sigmoid(i_r + h_r)
        z = _sigmoid(i_z + h_z)
        n = np.tanh(i_n + r * h_n)
        h = (1.0 - z) * n + z * h
        ys[t] = h
    return ys


def _conv1d_same(x, w):
    # cross-correlation with zero 'same' padding; x [B,T], w [k]
    k = w.shape[0]
    p = k // 2
    xp = np.pad(x, ((0, 0), (p, p)))
    out = np.zeros_like(x)
    for j in range(k):
        out += w[j] * xp[:, j : j + x.shape[1]]
    return out


_F8 = getattr(ml_dtypes, "float8_e4m3fn", None) or ml_dtypes.float8_e4m3


def _make_in_maps(inputs):
    fC = np.asarray(inputs["featContent"], np.float32)
    fD = np.asarray(inputs["featDistort"], np.float32)
    mF = np.asarray(inputs["motionFeat"], np.float32)
    fc0_w = np.asarray(inputs["fc0_w"], np.float32)
    # wH[p, ko, m] = fc0_w[m, ko*128+p]
    wH_np = np.ascontiguousarray(
        fc0_w.reshape(RED, KO, 128).transpose(2, 1, 0)
    ).astype(ml_dtypes.bfloat16)

    def build(c):
        sl = slice(c * BL, (c + 1) * BL)
        # xH[p, n, ko, c] = concat_feat[n*512+c (flat b*T+t), ko*128+p]
        # Values are quantized with e4m3fn (OCP) semantics -- probed to be
        # the HW's fp8e4 interpretation -- then bit-reinterpreted to the
        # ml_dtypes.float8_e4m3 container mybir.dt.np(float8e4) expects.
        xH = np.empty((128, NT, KO, 512), _F8)
        fc = fC[sl].reshape(BLT, D_CONTENT).astype(_F8)
        xH[:, :, 0:32, :] = fc.reshape(NT, 512, 32, 128).transpose(3, 0, 2, 1)
        fd = fD[sl].reshape(BLT, D_DISTORT).astype(_F8)
        xH[:, :, 32:36, :] = fd.reshape(NT, 512, 4, 128).transpose(3, 0, 2, 1)
        mf = mF[sl].reshape(BLT, D_MOTION).astype(_F8)
        xH[:, :, 36:38, :] = mf.reshape(NT, 512, 2, 128).transpose(3, 0, 2, 1)
        return {"xH": xH.view(ml_dtypes.float8_e4m3), "wH": wH_np}

    with ThreadPoolExecutor(N_CORES) as ex:
        return list(ex.map(build, range(N_CORES)))


def kernel(**inputs):
    inputLength = np.asarray(inputs["inputLength"])
    fc0_b = np.asarray(inputs["fc0_b"], np.float32)

    in_maps = _make_in_maps(inputs)
    results = _run_device(in_maps)

    scores = np.empty((B, T, RED), np.float32)
    for c in range(N_CORES):
        sT = results[c]["sT"]  # [RED, BLT]
        scores[c * BL : (c + 1) * BL] = (
            sT.T.reshape(BL, T, RED).astype(np.float32)
        )
    scores += fc0_b

    # BiGRU (fp32 host)
    x_tbd = scores.transpose(1, 0, 2)  # [T,B,RED]
    gi_f = x_tbd @ np.asarray(inputs["gru_wi_f"], np.float32).T + np.asarray(
        inputs["gru_bi_f"], np.float32
    )
    gi_b = x_tbd @ np.asarray(inputs["gru_wi_b"], np.float32).T + np.asarray(
        inputs["gru_bi_b"], np.float32
    )
    yf = _gru_dir(gi_f, np.asarray(inputs["gru_wh_f"], np.float32),
                  np.asarray(inputs["gru_bh_f"], np.float32), reverse=False)
    yb = _gru_dir(gi_b, np.asarray(inputs["gru_wh_b"], np.float32),
                  np.asarray(inputs["gru_bh_b"], np.float32), reverse=True)
    outputs = np.concatenate([yf, yb], -1).transpose(1, 0, 2)  # [B,T,2H]

    q_w = np.asarray(inputs["q_w"], np.float32)
    q_b = np.asarray(inputs["q_b"], np.float32)
    q = (outputs @ q_w.T + q_b)[..., 0]  # [B,T]

    lengths = inputLength.astype(np.int64) - 2 * (TIME_INTERVAL // 2) - 1
    mask = np.arange(T)[None, :] < lengths[:, None]
    qm = np.where(mask, q, 0.0).astype(np.float32)

    total = np.zeros((B,), np.float32)
    for wk in ("w1", "w2", "w3"):
        w = np.asarray(inputs[wk], np.float32)
        logits = np.where(mask, _conv1d_same(qm, w), NEG).astype(np.float32)
        m = logits.max(-1, keepdims=True)
        e = np.exp(logits - m)
        sm = e / e.sum(-1, keepdims=True)
        total = total + (sm * qm).sum(-1)
    return (total / 3.0)[:, None].astype(np.float32)
